# revision 1
# baseline (speedup 1.0000x reference)
"""Causal self-attention (B=4, T=2048, C=1024, H=16, D=64) on 8 TRN2 NeuronCores.

Sharding: batch x head-group. Core c handles batch b = c//2 and heads
hg*8..hg*8+8 where hg = c%2 (data parallel on batch, tensor parallel on heads;
w_qkv column-sharded, w_out row-sharded). Each core is fully independent; the
host sums the two per-batch partial outputs and adds the bias terms.

v2: bf16 everywhere (inputs, weights, activations; PSUM accumulation stays
fp32), single fused pipeline over the 4 query chunks ic:
  B(ic): QKV projection for t in [ic*512,(ic+1)*512): Q^T/K^T [qk-col, t]
    via lhsT=w1 rhs=xT, V [t, v-col] via lhsT=xT rhs=w1v, packed per t-tile
    as [128, 8*(64+1)] with a ones column per head (PV then also emits the
    softmax denominator).
  C(ic): per head: S^T [j, i] = K^T.T @ Q^T narrowed to the causal range;
    diagonal blocks masked on the PE by accumulating a -1e30 upper-triangle
    tile (matmul lhsT=identity rhs=BIGU) into the same PSUM group; exp on
    ScalarE straight out of PSUM (no max subtraction; scores O(1)).
    PV flipped: O[i, d|den] = lhsT=P^T-block @ rhs=[V|1] (65-col moving,
    both head halves packed per ops bank, single pending-zero group).
    Normalize = DVE reciprocal of the per-partition denominator column +
    tensor_scalar_mul, then DMA-xbar transpose of [128 i, 128 d] blocks
    into the O^T [d, t] layout phase D consumes.
  D(ic): y[t, c] = O^T.T @ w2 for the chunk's 4 t-tiles, y DMA'd out bf16.
B(ic+1) and D(ic-1) instructions are interleaved as fillers into C(ic)'s
group loop so the PE keeps working while ScalarE (the exp bottleneck)
drains; all engine queues are in-order so issue order fixes the schedule.
"""

import numpy as np

import concourse.bass as bass
import concourse.bacc as bacc
import concourse.mybir as mybir
from concourse.tile import TileContext

# ---- problem constants (hardcoded per contract) ----
B, T, C = 4, 2048, 1024
H_GLOBAL, D = 16, 64
HL = 8                      # local heads per core
N_CORES = 8
P = 128
KT_C = C // P               # 8 contraction tiles over C
NT = T // P                 # 16 t-tiles
IB = 512                    # query block (i-chunk)
NIC = T // IB               # 4 i-chunks
G = 2                       # j-tiles per exp group
M1 = 3 * HL * D             # 1536 local qkv cols
F32 = mybir.dt.float32
BF16 = mybir.dt.bfloat16
SCALE = 1.0 / np.sqrt(D).astype(np.float32)


class Filler:
    """FIFO of instruction-issuing thunks, drained into another loop."""

    def __init__(self):
        self.thunks = []

    def add(self, *fns):
        self.thunks.extend(fns)

    def emit(self, n):
        for _ in range(min(n, len(self.thunks))):
            self.thunks.pop(0)()

    def flush(self):
        while self.thunks:
            self.thunks.pop(0)()


def build_nc(repeat=1, phases="BCD", fake_exp=False):
    nc = bacc.Bacc("TRN2", target_bir_lowering=False)
    EXPF = mybir.ActivationFunctionType.Exp

    xT = nc.dram_tensor("xT", [C, T], BF16, kind="ExternalInput").ap()
    w1 = nc.dram_tensor("w1", [C, M1], BF16, kind="ExternalInput").ap()
    b1 = nc.dram_tensor("b1", [M1], F32, kind="ExternalInput").ap()
    w2 = nc.dram_tensor("w2", [HL * D, C], BF16, kind="ExternalInput").ap()
    y = nc.dram_tensor("y", [T, C], BF16, kind="ExternalOutput").ap()

    xT_r = xT.rearrange("(k p) t -> p k t", p=P)

    with TileContext(nc) as tc:
      for _rep in range(repeat):
        with tc.tile_pool(name="persist", bufs=1) as persist, \
             tc.tile_pool(name="xs", bufs=2) as xs, \
             tc.tile_pool(name="fpsum", bufs=2, space="PSUM") as fpsum, \
             tc.tile_pool(name="spsum", bufs=2, space="PSUM") as spool, \
             tc.tile_pool(name="opsum", bufs=2, space="PSUM") as opool, \
             tc.tile_pool(name="ptp", bufs=3) as ptp, \
             tc.tile_pool(name="recp", bufs=2) as recp, \
             tc.tile_pool(name="onrm", bufs=2) as onrm, \
             tc.tile_pool(name="yout", bufs=3) as yout:
            ones_bf = persist.tile([P, HL], BF16, tag="ones", name="ones_bf")
            nc.vector.memset(ones_bf[:, :], 1.0)
            # causal mask for diagonal S blocks, applied on the PE: one extra
            # matmul accumulates BIGU (-1e30 where col < partition, else 0)
            # into the diagonal PSUM block via an identity rhs; exp then
            # underflows to exact zeros. No per-block gpsimd/DVE op needed.
            bigu = persist.tile([P, P], BF16, tag="bigu", name="bigu")
            nc.vector.memset(bigu[:, :], 0.0)
            nc.gpsimd.affine_select(
                out=bigu[:, :], in_=bigu[:, :], pattern=[[1, P]],
                compare_op=mybir.AluOpType.is_ge, fill=-1e30,
                base=0, channel_multiplier=-1)
            iden = persist.tile([P, P], BF16, tag="iden", name="iden")
            nc.vector.memset(iden[:, :], 1.0)
            nc.gpsimd.affine_select(
                out=iden[:, :], in_=iden[:, :], pattern=[[1, P]],
                compare_op=mybir.AluOpType.is_ge, fill=0.0,
                base=0, channel_multiplier=-1)
            nc.gpsimd.affine_select(
                out=iden[:, :], in_=iden[:, :], pattern=[[-1, P]],
                compare_op=mybir.AluOpType.is_ge, fill=0.0,
                base=0, channel_multiplier=1)
            b1_sb = persist.tile([P, 12], F32, tag="b1", name="b1_sb")
            w2_sb = persist.tile([P, HL * D // P, C], BF16, tag="w2", name="w2_sb")
            w1_sb = persist.tile([P, KT_C, M1], BF16, tag="w1", name="w1_sb")

            # per-chunk persistent activations (bf16)
            QTc = [[persist.tile([P, IB], BF16, tag=f"QT{pr}_{tc_}",
                                 name=f"QT{pr}_{tc_}")
                    for tc_ in range(NIC)] for pr in range(HL // 2)]
            KTc = [[persist.tile([P, IB], BF16, tag=f"KT{pr}_{tc_}",
                                 name=f"KT{pr}_{tc_}")
                    for tc_ in range(NIC)] for pr in range(HL // 2)]
            V = [persist.tile([P, HL * (D + 1)], BF16, tag=f"V{tt}",
                              name=f"V{tt}")
                 for tt in range(NT)]
            OPc = [[persist.tile([P, IB], BF16, tag=f"OP{pr}_{tc_}",
                                 name=f"OP{pr}_{tc_}")
                    for tc_ in range(NIC)] for pr in range(HL // 2)]

            # ---------------- phase B thunks (QKV projection chunk) --------
            def b_chunk_thunks(tc_, fill):
                xc_cell = {}

                def dma_xc(tc_=tc_):
                    xc = xs.tile([P, KT_C, IB], BF16, tag="xc", name="xc")
                    nc.sync.dma_start(out=xc[:, 0:4, :],
                                      in_=xT_r[:, 0:4, tc_ * IB:(tc_ + 1) * IB])
                    nc.scalar.dma_start(out=xc[:, 4:8, :],
                                        in_=xT_r[:, 4:8,
                                                 tc_ * IB:(tc_ + 1) * IB])
                    xc_cell["t"] = xc
                fill.add(dma_xc)

                for ttl in range(IB // P):
                    tt = tc_ * (IB // P) + ttl
                    vp_cell = {}
                    for k in range(KT_C):
                        def mm(k=k, ttl=ttl, vp_cell=vp_cell):
                            if k == 0:
                                vp_cell["t"] = fpsum.tile([P, HL * D], F32,
                                                          tag="fp", name="vp")
                            nc.tensor.matmul(
                                vp_cell["t"][:, :],
                                lhsT=xc_cell["t"][:, k, ttl * P:(ttl + 1) * P],
                                rhs=w1_sb[:, k, 2 * HL * D:3 * HL * D],
                                start=(k == 0), stop=(k == KT_C - 1))
                        fill.add(mm)

                    def evac(tt=tt, vp_cell=vp_cell):
                        nc.vector.tensor_copy(
                            out=V[tt].rearrange(
                                "p (h x) -> p h x", x=D + 1)[:, :, 0:D],
                            in_=vp_cell["t"].rearrange("p (h x) -> p h x", x=D))
                        nc.vector.tensor_copy(
                            out=V[tt].rearrange(
                                "p (h x) -> p h x", x=D + 1)[:, :, D:D + 1],
                            in_=ones_bf.rearrange("p (h o) -> p h o", o=1))
                    fill.add(evac)

                for pr in range(HL // 2):
                    for qk in range(2):
                        qp_cell = {}
                        for k in range(KT_C):
                            def mm(k=k, pr=pr, qk=qk, qp_cell=qp_cell):
                                if k == 0:
                                    qp_cell["t"] = fpsum.tile(
                                        [P, IB], F32, tag="fp", name="qp")
                                nc.tensor.matmul(
                                    qp_cell["t"][:, :],
                                    lhsT=w1_sb[:, k,
                                               qk * HL * D + pr * P:
                                               qk * HL * D + (pr + 1) * P],
                                    rhs=xc_cell["t"][:, k, :],
                                    start=(k == 0), stop=(k == KT_C - 1))
                            fill.add(mm)

                        def evac(pr=pr, qk=qk, tc_=tc_, qp_cell=qp_cell):
                            dst = (QTc if qk == 0 else KTc)[pr][tc_]
                            nc.vector.tensor_scalar_add(
                                dst[:, :], qp_cell["t"][:, :],
                                b1_sb[:, 4 * qk + pr:4 * qk + pr + 1])
                        fill.add(evac)

            # ---------------- phase D thunks (out projection chunk) --------
            def d_chunk_thunks(tc_, fill, tail=False):
                for ttl in range(IB // P):
                    for cc in range(C // IB):
                        yp_cell = {}
                        for pr in range(HL // 2):
                            def mm(pr=pr, ttl=ttl, cc=cc, tc_=tc_,
                                   yp_cell=yp_cell):
                                if pr == 0:
                                    yp_cell["t"] = fpsum.tile(
                                        [P, IB], F32, tag="fp", name="yp")
                                nc.tensor.matmul(
                                    yp_cell["t"][:, :],
                                    lhsT=OPc[pr][tc_][:, ttl * P:(ttl + 1) * P],
                                    rhs=w2_sb[:, pr, cc * IB:(cc + 1) * IB],
                                    start=(pr == 0), stop=(pr == HL // 2 - 1))
                            fill.add(mm)

                        def evac(ttl=ttl, cc=cc, tc_=tc_, yp_cell=yp_cell):
                            tt = tc_ * (IB // P) + ttl
                            ysb = yout.tile([P, IB], BF16, tag="ysb",
                                            name="ysb")
                            nc.vector.tensor_copy(out=ysb[:, :],
                                                  in_=yp_cell["t"][:, :])
                            (nc.scalar if tail else nc.sync).dma_start(
                                out=y[tt * P:(tt + 1) * P,
                                      cc * IB:(cc + 1) * IB],
                                in_=ysb[:, :])
                        fill.add(evac)

            # ---------------- fused pipeline over i-chunks -----------------
            fill = Filler()
            b_chunk_thunks(0, fill)
            fill.emit(1)        # xc(0) DMA first in the in-order SP queue
            nc.sync.dma_start(out=b1_sb[:, :],
                              in_=b1.rearrange("(m p) -> p m", p=P))
            # w1 k-tiles alternate between the two hwdge queues (SP and
            # Activation -- ScalarE is idle through phase B) to halve the
            # prologue weight-load latency
            for k in range(KT_C):
                eng = nc.sync if k % 2 == 0 else nc.scalar
                eng.dma_start(out=w1_sb[:, k, :],
                              in_=w1.rearrange("(k p) m -> p k m",
                                               p=P)[:, k, :])
            nc.scalar.dma_start(out=w2_sb[:, :, :],
                                in_=w2.rearrange("(k p) c -> p k c", p=P))
            fill.flush()

            for ic in range(NIC):
                # queue fillers: xc prefetch first, then previous D chunk
                # interleaved with the next B chunk
                fa, fb = Filler(), Filler()
                if "D" in phases and ic > 0:
                    d_chunk_thunks(ic - 1, fa)
                if ic + 1 < NIC:
                    b_chunk_thunks(ic + 1, fb)
                fb.emit(1)      # xc(ic+1) DMA prefetch
                while fa.thunks or fb.thunks:
                    if fa.thunks:
                        fill.add(fa.thunks.pop(0))
                    if fb.thunks:
                        fill.add(fb.thunks.pop(0))

                if "C" in phases:
                    njt = (ic + 1) * (IB // P)
                    for pr in range(HL // 2):
                        # O accumulators in [i, d|den] layout: per i-tile a
                        # 65-wide block (64 dims + softmax denominator)
                        ops = [opool.tile([P, 4 * (D + 1)], F32, tag="op",
                                          name=f"o{h2}") for h2 in range(2)]
                        onorm = onrm.tile([P, IB], BF16, tag="on",
                                          name="onorm")
                        ngr = (njt + G - 1) // G
                        for g in range(ngr):
                            jts = list(range(g * G, min((g + 1) * G, njt)))
                            rel0 = max(0, jts[0] * P - ic * IB)
                            for h2 in range(2):
                                hs = h2 * D
                                h = pr * 2 + h2
                                sp = spool.tile([P, G * IB], F32, tag="sp",
                                                name="sp")
                                srels = [max(0, jt * P - ic * IB)
                                         for jt in jts]
                                for jl, jt in enumerate(jts):
                                    rel = jt * P - ic * IB
                                    diag = rel >= 0
                                    srel = srels[jl]
                                    nc.tensor.matmul(
                                        sp[:, jl * IB + srel:(jl + 1) * IB],
                                        lhsT=KTc[pr][jt // 4][
                                            hs:hs + D,
                                            (jt % 4) * P:(jt % 4 + 1) * P],
                                        rhs=QTc[pr][ic][
                                            hs:hs + D, srel:IB],
                                        start=True, stop=not diag)
                                    if diag:  # add -1e30 above the diagonal
                                        nc.tensor.matmul(
                                            sp[:, jl * IB + rel:
                                               jl * IB + rel + P],
                                            lhsT=iden[:, :],
                                            rhs=bigu[:, :],
                                            start=False, stop=True)
                                pt = ptp.tile([P, G * IB], BF16, tag="pt",
                                              name="pt")
                                if fake_exp:
                                    nc.vector.tensor_copy(
                                        out=pt[:, 0:len(jts) * IB],
                                        in_=sp[:, 0:len(jts) * IB])
                                elif not any(srels):
                                    nc.scalar.activation(
                                        pt[:, 0:len(jts) * IB],
                                        sp[:, 0:len(jts) * IB],
                                        EXPF, scale=float(SCALE))
                                else:
                                    # written regions have holes between the
                                    # tiles; exp each tile's columns alone
                                    for jl in range(len(jts)):
                                        nc.scalar.activation(
                                            pt[:, jl * IB + srels[jl]:
                                               (jl + 1) * IB],
                                            sp[:, jl * IB + srels[jl]:
                                               (jl + 1) * IB],
                                            EXPF, scale=float(SCALE))
                                # single accumulation group per ops tile: the
                                # first matmul's start marks the whole 2KB
                                # zero region pending; each region's first
                                # write overwrites, later ones accumulate
                                for jl, jt in enumerate(jts):
                                    for it in range(IB // P):
                                        git = ic * (IB // P) + it
                                        if jt > git:
                                            continue
                                        nc.tensor.matmul(
                                            ops[h2][:, it * (D + 1):
                                                    (it + 1) * (D + 1)],
                                            lhsT=pt[:, jl * IB + it * P:
                                                    jl * IB + (it + 1) * P],
                                            rhs=V[jt][:, h * (D + 1):
                                                      (h + 1) * (D + 1)],
                                            start=(jt == 0 and it == 0),
                                            stop=(jt == njt - 1 and it == 3))
                                fill.emit(2)
                        # normalize per i-partition: O[i, d] * recip(den[i])
                        # written into onorm [i, it*(d0|d1)], then transposed
                        # per 128x128 block into OPc [d, i] by the DMA xbar.
                        for h2 in range(2):
                            osb = recp.tile([P, 4 * (D + 1)], F32, tag="osb",
                                            name="osb")
                            nc.vector.tensor_copy(out=osb[:, :],
                                                  in_=ops[h2][:, :])
                            den = osb.rearrange("p (i x) -> p i x",
                                                x=D + 1)[:, :, D:D + 1]
                            nc.vector.reciprocal(den, den)
                            for it in range(IB // P):
                                nc.vector.tensor_scalar_mul(
                                    onorm[:, it * P + h2 * D:
                                          it * P + (h2 + 1) * D],
                                    osb[:, it * (D + 1):it * (D + 1) + D],
                                    osb[:, it * (D + 1) + D:
                                        (it + 1) * (D + 1)])
                            fill.emit(2)
                        for it in range(IB // P):
                            nc.sync.dma_start_transpose(
                                out=OPc[pr][ic][:, it * P:(it + 1) * P],
                                in_=onorm[:, it * P:(it + 1) * P])
                        fill.emit(2)
                fill.flush()

            if "D" in phases:
                d_chunk_thunks(NIC - 1, fill, tail=True)
                fill.flush()
    nc.compile()
    return nc


_NC_CACHE = None


def _get_nc():
    global _NC_CACHE
    if _NC_CACHE is None:
        _NC_CACHE = build_nc()
    return _NC_CACHE


def shard_inputs(x, w_qkv, b_qkv, w_out):
    """Build the 8 per-core input maps (bf16 matmul inputs)."""
    import ml_dtypes
    bf = ml_dtypes.bfloat16
    x = np.asarray(x, dtype=np.float32)
    w_qkv = np.asarray(w_qkv, dtype=np.float32)
    b_qkv = np.asarray(b_qkv, dtype=np.float32)
    w_out = np.asarray(w_out, dtype=np.float32)
    in_maps = []
    for core in range(N_CORES):
        b, hg = core // 2, core % 2
        cs = hg * HL * D              # 512-wide contiguous head-group slice
        w1 = np.ascontiguousarray(np.concatenate(
            [w_qkv[:, cs:cs + HL * D],
             w_qkv[:, C + cs:C + cs + HL * D],
             w_qkv[:, 2 * C + cs:2 * C + cs + HL * D]], axis=1))
        b1 = np.ascontiguousarray(np.concatenate(
            [b_qkv[cs:cs + HL * D],
             b_qkv[C + cs:C + cs + HL * D],
             b_qkv[2 * C + cs:2 * C + cs + HL * D]]))
        in_maps.append({
            "xT": np.ascontiguousarray(x[b].T).astype(bf),
            "w1": w1.astype(bf),
            "b1": b1,
            "w2": np.ascontiguousarray(w_out[cs:cs + HL * D, :]).astype(bf),
        })
    return in_maps


def combine_outputs(results, b_qkv, w_out, b_out):
    """Sum per-batch partials from the two head-group cores + bias terms."""
    bias_vec = (np.asarray(b_qkv[2 * C:3 * C], dtype=np.float32) @
                np.asarray(w_out, dtype=np.float32) +
                np.asarray(b_out, dtype=np.float32))
    y = np.empty((B, T, C), dtype=np.float32)
    for b in range(B):
        y[b] = (np.asarray(results[2 * b]["y"], dtype=np.float32) +
                np.asarray(results[2 * b + 1]["y"], dtype=np.float32) +
                bias_vec)
    return y


def kernel(x, w_qkv, b_qkv, w_out, b_out, *, trace=False, _sink=None):
    from concourse.bass_utils import run_bass_kernel_spmd
    nc = _get_nc()
    in_maps = shard_inputs(x, w_qkv, b_qkv, w_out)
    res = run_bass_kernel_spmd(nc, in_maps, core_ids=list(range(N_CORES)),
                               trace=trace)
    if _sink is not None:
        _sink.append(res)
    return combine_outputs(res.results, b_qkv, w_out, b_out)

